# revision 1
# baseline (speedup 1.0000x reference)
"""Differential multi-head self-attention on 8 Trainium2 NeuronCores, v2.

Sharding: core c handles batch b = c // 4 and heads {2*(c%4), 2*(c%4)+1}.
Host pre-transposes x (xT = x[b].T, bf16) and pre-folds rms_w and the
(1 - lambda_init) factor into Wo; the host sums per-head partial output
projections and adds bo.

Device math per (b, h), all matmuls bf16 (1 cycle/row):
  KT/QT = W.T @ xT, V = xT.T @ Wv
  per q tile of 256 (k-pair-major, [128,1024] PSUM score tiles):
    S[k, q]   = K @ Q.T for both halves  (PE)
    E = exp(S/sqrt(half)) (ACT, bf16, both halves + 2 k-chunks per instr)
    mask via keep-pattern multiply       (DVE, diagonal pair only)
    sums s1,s2 via N=1 matmuls with ones (PE, nearly free)
  c[q] = lam * s1[q]/s2[q]  ->  transposed to a row (PE) and broadcast
    to all partitions (GPSIMD partition_broadcast)
  combine in SBUF: Ec = E1 - c*E2       (DVE, in-place, one pass)
  AV: O[q, d] = Ec.T @ V                (PE, single pass = half of v1)
  g[q] = rsqrt(mean_d(O^2) + eps*s1^2)  (ACT square-accum + ln/exp)
  osb = O * g  ->  transpose (PE)  ->  out = osb.T @ Wo'  -> DMA

Software pipeline per iteration t (one q tile):
  tailPE(t-2) | combines(t-1) on DVE | scores(t) with AV(t-1) interleaved
  on PE | c-chain(t) | tailpre(t-1).
"""

import numpy as np
import ml_dtypes
from collections import deque

import concourse.bass as bass
import concourse.mybir as mybir
import concourse.tile as tile
from concourse import bacc
from concourse.bass_utils import run_bass_kernel_spmd
from concourse.hw_specs import get_activation_tables
from concourse.masks import make_identity

B, S, E, H, D = 2, 2048, 512, 8, 512
HALF = D // 2
HLOC = 2            # heads per core
NCORES = 8
QT = 256            # q tile
NQT = S // QT       # 8
KC = 128            # k chunk
NKC = S // KC       # 16
NKP = NKC // 2      # k pairs
NQC = QT // 128     # 2
NDC = D // 128      # 4
NEC = E // 128      # 4
SCALE = 1.0 / float(np.sqrt(HALF))
EPS = float(np.finfo(np.float32).eps)
LAMBDA_INIT = 0.8

f32 = mybir.dt.float32
f32r = mybir.dt.float32r
bf16 = mybir.dt.float16  # fp16: same PE speed as bf16, 8x less rounding
AF = mybir.ActivationFunctionType
ALU = mybir.AluOpType

SKIP, FULL = -1, -2


def _analyze_mask(mask):
    """Per (q-tile, k-chunk) status: SKIP / FULL / keep-pattern index.

    A pattern is [128, 512]: the [128 k, 256 q] keep matrix duplicated
    across both halves.
    """
    status = [[SKIP] * NKC for _ in range(NQT)]
    pats = []
    pat_idx = {}
    for t in range(NQT):
        for kc in range(NKC):
            blk = mask[t * QT:(t + 1) * QT, kc * 128:(kc + 1) * 128]  # [256q, 128k]
            if blk.all():
                status[t][kc] = SKIP
            elif not blk.any():
                status[t][kc] = FULL
            else:
                keep = (~blk).T.astype(np.float32)  # [128, 256]
                pat = np.concatenate([keep, keep], axis=1)  # [128, 512]
                key = pat.tobytes()
                if key not in pat_idx:
                    pat_idx[key] = len(pats)
                    pats.append(pat)
                status[t][kc] = pat_idx[key]
    return status, pats


def _build(status, npat, repeat=1, unroll=1):  # noqa: C901
    nc = bacc.Bacc("TRN2", target_bir_lowering=False, debug=False)

    xt_d = nc.dram_tensor("xt", [E, S], bf16, kind="ExternalInput")
    wq_d = nc.dram_tensor("wq", [HLOC, E, D], bf16, kind="ExternalInput")
    wk_d = nc.dram_tensor("wk", [HLOC, E, D], bf16, kind="ExternalInput")
    wv_d = nc.dram_tensor("wv", [HLOC, E, D], bf16, kind="ExternalInput")
    wo_d = nc.dram_tensor("wo", [HLOC, D, E], bf16, kind="ExternalInput")
    lamneg_d = nc.dram_tensor("lamneg", [HLOC, 128, 1], f32, kind="ExternalInput")
    keeps_d = nc.dram_tensor("keeps", [max(npat, 1), 128, 512], bf16,
                             kind="ExternalInput")
    out_d = nc.dram_tensor("out", [HLOC, S, E], f32, kind="ExternalOutput")
    iters_d = nc.dram_tensor("iters", [1, 1], f32, kind="ExternalOutput") if repeat > 1 else None

    act_sets = list(get_activation_tables(nc.m.arch).keys())
    nle_set = act_sets.index("natural_log_exp_and_others")

    with tile.TileContext(nc) as tc:
        with tc.tile_pool(name="cst", bufs=1) as cst, \
             tc.tile_pool(name="big", bufs=1) as big, \
             tc.tile_pool(name="big2", bufs=2) as big2, \
             tc.tile_pool(name="epool", bufs=2) as epool, \
             tc.tile_pool(name="wts", bufs=2) as wts, \
             tc.tile_pool(name="scr", bufs=2) as scr, \
             tc.tile_pool(name="ps", bufs=3, space="PSUM") as ps, \
             tc.tile_pool(name="pst", bufs=2, space="PSUM") as pst, \
             tc.tile_pool(name="pso", bufs=1, space="PSUM") as pso, \
             tc.tile_pool(name="pss", bufs=1, space="PSUM") as pss:

            nc.scalar.add_instruction(mybir.InstLoadActFuncSet(
                name=nc.get_next_instruction_name(),
                ins=[], outs=[], act_func_set_id=nle_set))

            ident = cst.tile([128, 128], bf16, tag="ident")
            make_identity(nc, ident[:])
            ones_bf = cst.tile([128, 1], bf16, tag="ones")
            nc.gpsimd.memset(ones_bf[:], 1.0)
            keeps_t = cst.tile([128, max(npat, 1), 512], bf16, tag="keeps")
            for i in range(npat):
                nc.sync.dma_start(keeps_t[:, i, :], keeps_d.ap()[i])
            lam_t = cst.tile([128, HLOC], f32, tag="lam")
            for h in range(HLOC):
                nc.sync.dma_start(lam_t[:, h:h + 1], lamneg_d.ap()[h])

            # xT[e, s]: straight DMA (host pre-transposed)
            xT = big.tile([128, NEC, S], bf16, tag="xT")
            nc.sync.dma_start(
                xT[:], xt_d.ap().rearrange("(a p) s -> p a s", p=128))

            if repeat > 1:
                ctr = cst.tile([1, 1], f32, tag="ctr")
                nc.gpsimd.memset(ctr[:], 0.0)
            rep_ctx = tc.For_i(0, repeat, 1) if repeat > 1 else None
            if rep_ctx is not None:
                rep_ctx.__enter__()
                nc.vector.tensor_scalar_add(ctr[:], ctr[:], 1.0)

            def emit_body():
                comb_q = deque()      # jobs awaiting c-chain-PE + combine + AV
                tailpe_q = deque()    # jobs awaiting transpose/outproj/DMA

                def emit_tailpe(j):
                    h = j["h"]
                    for qc in range(NQC):
                        tp_g = pst.tile([128, 512], bf16, tag="tail",
                                        name="tp_g")
                        for dc in range(NDC):
                            nc.tensor.transpose(
                                tp_g[:, dc * 128:(dc + 1) * 128],
                                j["osb"][qc][:, dc * 128:(dc + 1) * 128],
                                ident[:])
                        ot_t = scr.tile([128, NDC, 128], bf16, tag=f"ot{qc}")
                        nc.scalar.activation(ot_t[:], tp_g[:], AF.Copy)
                        out_ps = pst.tile([128, E], f32, tag="tail",
                                          name="out_ps")
                        for dc in range(NDC):
                            nc.tensor.matmul(
                                out_ps[:], ot_t[:, dc, :], j["wo"][:, dc, :],
                                start=(dc == 0), stop=(dc == NDC - 1))
                        out_sb = scr.tile([128, E], f32, tag="outsb")
                        nc.vector.tensor_copy(out_sb[:], out_ps[:])
                        q0 = j["t"] * QT + qc * 128
                        nc.sync.dma_start(out_d.ap()[h, q0:q0 + 128, :], out_sb[:])

                def emit_cchain_pe(j):
                    # ct transposes + partition broadcast for tile t-1;
                    # rec/cneg were computed at the end of the prior iteration
                    cs_t = scr.tile([1, QT], bf16, tag="cs")
                    for qc in range(NQC):
                        ct_ps = pst.tile([1, 128], bf16, tag="tail",
                                         name="ct_ps")
                        nc.tensor.transpose(
                            ct_ps[:], j["cneg"][:, qc:qc + 1], ident[:])
                        nc.vector.tensor_copy(
                            cs_t[:, qc * 128:(qc + 1) * 128], ct_ps[:])
                    cb = scr.tile([128, 1, QT], bf16, tag="cb")
                    nc.gpsimd.partition_broadcast(cb[:, 0, :], cs_t[:])
                    j["cb"] = cb

                def emit_combines(j):
                    Et, cb = j["E"], j["cb"]
                    nkc = len(j["kcs"])
                    for p in range(nkc // 2):
                        e2 = Et[:, 2 * p:2 * p + 2, 256:512]
                        nc.vector.tensor_tensor(
                            out=e2, in0=e2,
                            in1=cb[:].to_broadcast([128, 2, 256]), op=ALU.mult)
                        nc.vector.tensor_tensor(
                            out=Et[:, 2 * p:2 * p + 2, 0:256],
                            in0=Et[:, 2 * p:2 * p + 2, 0:256],
                            in1=e2, op=ALU.add)

                def emit_av_chunk(j, i):
                    Et, Vt = j["E"], j["V"]
                    kc = j["kcs"][i]
                    nkc = len(j["kcs"])
                    for qc in range(NQC):
                        nc.tensor.matmul(
                            j["o_ps"][:, qc * 512:(qc + 1) * 512],
                            Et[:, kc, qc * 128:(qc + 1) * 128],
                            Vt[:, kc, :],
                            start=(i == 0), stop=(i == nkc - 1))

                def emit_tailpre(j):
                    ms = scr.tile([128, NQC], f32, tag="ms")
                    lnm = scr.tile([128, NQC], f32, tag="lnm")
                    g = scr.tile([128, NQC], f32, tag="g")
                    osbs = []
                    for qc in range(NQC):
                        osq = scr.tile([128, D], bf16, tag="osq", bufs=1)
                        nc.scalar.activation(
                            osq[:], j["o_ps"][:, qc * 512:(qc + 1) * 512],
                            AF.Square, scale=float(1.0 / np.sqrt(D)),
                            accum_out=ms[:, qc:qc + 1])
                        nc.scalar.activation(
                            lnm[:, qc:qc + 1], ms[:, qc:qc + 1], AF.Ln,
                            bias=j["s1e"][:, qc:qc + 1])
                        nc.scalar.activation(
                            g[:, qc:qc + 1], lnm[:, qc:qc + 1], AF.Exp,
                            scale=-0.5)
                        osb = scr.tile([128, D], bf16, tag=f"osb{qc}")
                        nc.vector.tensor_scalar_mul(
                            osb[:], j["o_ps"][:, qc * 512:(qc + 1) * 512],
                            g[:, qc:qc + 1])
                        osbs.append(osb)
                    j["osb"] = osbs

                def emit_iteration(hstate, t):
                    j = comb_q.popleft() if comb_q else None
                    if j is not None:
                        emit_combines(j)
                        j["o_ps"] = pso.tile([128, 2 * D], f32, tag="ops", name="o_ps")
                    nj = len(j["kcs"]) if j is not None else 0
                    popped_tail = False

                    if t is not None:
                        h = hstate["h"]
                        st = hstate["status"][t]
                        kcs = [kc for kc in range(NKC) if st[kc] != SKIP]
                        nkc = len(kcs)
                        q0 = t * QT
                        KTt, QTt = hstate["KT"], hstate["QT"]

                        Et = epool.tile([128, NKC, 512], bf16, tag="E")
                        sums_ps = pss.tile([128, 2 * NQC], f32, tag="sums")

                        def emit_sums(i):
                            # single PSUM accumulation group across all four
                            # (half, qc) columns: they share one zero region
                            kc = kcs[i]
                            for half in (0, 1):
                                for qc in range(NQC):
                                    nc.tensor.matmul(
                                        sums_ps[:, half * NQC + qc:
                                                half * NQC + qc + 1],
                                        Et[:, kc,
                                           half * 256 + qc * 128:
                                           half * 256 + qc * 128 + 128],
                                        ones_bf[:],
                                        start=(i == 0 and half == 0 and
                                               qc == 0),
                                        stop=(i == nkc - 1 and half == 1 and
                                              qc == NQC - 1))

                        for i, kc in enumerate(kcs):
                            sps = ps.tile([128, 512], f32, tag="mm",
                                          name="sps")
                            for half in (0, 1):
                                for jdc in (0, 1):
                                    dc = half * 2 + jdc
                                    nc.tensor.matmul(
                                        sps[:, half * 256:half * 256 + 256],
                                        KTt[:, dc, kc * 128:(kc + 1) * 128],
                                        QTt[:, dc, q0:q0 + QT],
                                        start=(jdc == 0), stop=(jdc == 1))
                            nc.scalar.activation(
                                Et[:, kc, :], sps[:], AF.Exp, scale=SCALE)
                            pat = st[kc]
                            if pat >= 0:
                                nc.vector.tensor_tensor(
                                    out=Et[:, kc, :], in0=Et[:, kc, :],
                                    in1=keeps_t[:, pat, :], op=ALU.mult)
                            if i > 0:
                                emit_sums(i - 1)
                            if j is not None and 1 <= i <= nj:
                                emit_av_chunk(j, i - 1)
                            if tailpe_q and i == min(3, nkc - 1):
                                emit_tailpe(tailpe_q.popleft())
                                popped_tail = True
                        emit_sums(nkc - 1)
                        # rec/cneg right away (DVE) while PE drains AVs
                        rec = scr.tile([128, NQC], f32, tag="rec")
                        nc.vector.reciprocal(rec[:], sums_ps[:, NQC:2 * NQC])
                        cneg = scr.tile([128, NQC], bf16, tag="cneg")
                        nc.vector.scalar_tensor_tensor(
                            out=cneg[:], in0=sums_ps[:, 0:NQC],
                            scalar=lam_t[:, h:h + 1],
                            in1=rec[:], op0=ALU.mult, op1=ALU.mult)
                        s1e = scr.tile([128, NQC], f32, tag="s1e")
                        nc.scalar.activation(
                            s1e[:], sums_ps[:, 0:NQC], AF.Square,
                            scale=float(np.sqrt(EPS)))
                        if j is not None:
                            for i2 in range(max(0, min(nkc - 1, nj)), nj):
                                emit_av_chunk(j, i2)

                        job = dict(
                            h=h, t=t, E=Et, V=hstate["V"], wo=hstate["wo"],
                            cneg=cneg, s1e=s1e, kcs=kcs)
                        emit_cchain_pe(job)
                        comb_q.append(job)
                    else:
                        if tailpe_q:
                            emit_tailpe(tailpe_q.popleft())
                            popped_tail = True
                        if j is not None:
                            for i2 in range(nj):
                                emit_av_chunk(j, i2)

                    if tailpe_q and not popped_tail:
                        emit_tailpe(tailpe_q.popleft())
                    if j is not None:
                        emit_tailpre(j)
                        tailpe_q.append(j)

                for h in range(HLOC):
                    wq_t = wts.tile([128, NEC, D], bf16, tag="wq")
                    wk_t = wts.tile([128, NEC, D], bf16, tag="wk")
                    wv_t = wts.tile([128, NEC, D], bf16, tag="wv")
                    wo_t = wts.tile([128, NDC, E], bf16, tag="wo")
                    for ec in range(NEC):
                        nc.sync.dma_start(wq_t[:, ec, :], wq_d.ap()[h, ec * 128:(ec + 1) * 128, :])
                        nc.sync.dma_start(wk_t[:, ec, :], wk_d.ap()[h, ec * 128:(ec + 1) * 128, :])
                        nc.sync.dma_start(wv_t[:, ec, :], wv_d.ap()[h, ec * 128:(ec + 1) * 128, :])
                    for dc in range(NDC):
                        nc.sync.dma_start(wo_t[:, dc, :], wo_d.ap()[h, dc * 128:(dc + 1) * 128, :])

                    # KT[d, k] = Wk.T @ xT ; QT[d, q] = Wq.T @ xT
                    KTt = big.tile([128, NDC, S], bf16, tag="KT")
                    QTt = big.tile([128, NDC, S], bf16, tag="QT")
                    for (w_t, dst, cp) in ((wk_t, KTt, "k"), (wq_t, QTt, "q")):
                        for dc in range(NDC):
                            for kt in range(S // 512):
                                kps = ps.tile([128, 512], f32, tag="mm",
                                              name="kps")
                                for ec in range(NEC):
                                    nc.tensor.matmul(
                                        kps[:],
                                        w_t[:, ec, dc * 128:(dc + 1) * 128],
                                        xT[:, ec, kt * 512:(kt + 1) * 512],
                                        start=(ec == 0), stop=(ec == NEC - 1))
                                if (dc + kt) % 2 == 0:
                                    nc.vector.tensor_copy(
                                        dst[:, dc, kt * 512:(kt + 1) * 512],
                                        kps[:])
                                else:
                                    nc.scalar.activation(
                                        dst[:, dc, kt * 512:(kt + 1) * 512],
                                        kps[:], AF.Copy)

                    # V[s, d] = x @ Wv
                    Vt = big2.tile([128, NKC, D], bf16, tag="V")
                    for sc in range(NKC):
                        vps = ps.tile([128, 512], f32, tag="mm", name="vps")
                        for ec in range(NEC):
                            nc.tensor.matmul(
                                vps[:],
                                xT[:, ec, sc * 128:(sc + 1) * 128],
                                wv_t[:, ec, :],
                                start=(ec == 0), stop=(ec == NEC - 1))
                        nc.scalar.activation(Vt[:, sc, :], vps[:], AF.Copy)

                    hstate = dict(h=h, status=status, KT=KTt, QT=QTt, V=Vt,
                                  wo=wo_t)
                    for t in range(NQT):
                        emit_iteration(hstate, t)
                emit_iteration(None, None)
                emit_iteration(None, None)

            for _u in range(unroll):
                emit_body()

            if rep_ctx is not None:
                rep_ctx.__exit__(None, None, None)
                nc.sync.dma_start(iters_d.ap()[:], ctr[:])

    nc.compile()
    return nc


_CACHE = {}


def _get_program(mask, repeat=1, unroll=1):
    key = (mask.tobytes(), repeat, unroll)
    if key not in _CACHE:
        status, pats = _analyze_mask(mask)
        nc = _build(status, len(pats), repeat=repeat, unroll=unroll)
        _CACHE[key] = (nc, pats)
    return _CACHE[key]


def make_in_maps(x, mask, Wq, bq, Wk, bk, Wv, bv, lq1, lk1, lq2, lk2,
                 lam_init_p, rms_w, Wo, bo, repeat=1, unroll=1):
    x = np.asarray(x, np.float32)
    mask = np.asarray(mask, bool)
    Wq = np.asarray(Wq, np.float32)
    Wk = np.asarray(Wk, np.float32)
    Wv = np.asarray(Wv, np.float32)
    Wo = np.asarray(Wo, np.float32)
    for b_ in (bq, bk, bv):
        assert np.abs(np.asarray(b_)).max() == 0.0, "nonzero qkv bias unsupported"
    lam = (np.exp((np.asarray(lq1, np.float32) * np.asarray(lk1, np.float32)).sum(-1))
           - np.exp((np.asarray(lq2, np.float32) * np.asarray(lk2, np.float32)).sum(-1))
           + np.asarray(lam_init_p, np.float32))  # [H]
    woF = Wo.reshape(H, D, E) * ((1.0 - LAMBDA_INIT) * np.asarray(rms_w, np.float32))[:, :, None]

    nc, pats = _get_program(mask, repeat=repeat, unroll=unroll)
    if pats:
        keeps = np.stack(pats).astype(np.float16)
    else:
        keeps = np.zeros((1, 128, 512), np.float16)

    in_maps = []
    for c in range(NCORES):
        b = c // 4
        h0 = HLOC * (c % 4)
        lamneg = np.repeat((-lam[h0:h0 + HLOC]).astype(np.float32)[:, None, None], 128, axis=1)
        in_maps.append({
            "xt": np.ascontiguousarray(x[b].T).astype(np.float16),
            "wq": np.ascontiguousarray(Wq[h0:h0 + HLOC]).astype(np.float16),
            "wk": np.ascontiguousarray(Wk[h0:h0 + HLOC]).astype(np.float16),
            "wv": np.ascontiguousarray(Wv[h0:h0 + HLOC]).astype(np.float16),
            "wo": np.ascontiguousarray(woF[h0:h0 + HLOC]).astype(np.float16),
            "lamneg": np.ascontiguousarray(lamneg),
            "keeps": keeps,
        })
    return nc, in_maps


def gather(results, bo):
    out = np.zeros((B, S, E), np.float32)
    for c in range(NCORES):
        out[c // 4] += results[c]["out"].sum(axis=0)
    out += np.asarray(bo, np.float32)[None, None, :]
    return out


def kernel(**inputs):
    nc, in_maps = make_in_maps(**inputs)
    res = run_bass_kernel_spmd(nc, in_maps, core_ids=list(range(NCORES)))
    return gather(res.results, inputs["bo"])



# revision 7
# speedup vs baseline: 1.0531x; 1.0531x over previous
"""Differential multi-head self-attention on 8 Trainium2 NeuronCores, v2.

Sharding: core c handles batch b = c // 4 and heads {2*(c%4), 2*(c%4)+1}.
Host pre-transposes x (xT = x[b].T, bf16) and pre-folds rms_w and the
(1 - lambda_init) factor into Wo; the host sums per-head partial output
projections and adds bo.

Device math per (b, h), all matmuls bf16 (1 cycle/row):
  KT/QT = W.T @ xT, V = xT.T @ Wv
  per q tile of 256 (k-pair-major, [128,1024] PSUM score tiles):
    S[k, q]   = K @ Q.T for both halves  (PE)
    E = exp(S/sqrt(half)) (ACT, bf16, both halves + 2 k-chunks per instr)
    mask via keep-pattern multiply       (DVE, diagonal pair only)
    sums s1,s2 via N=1 matmuls with ones (PE, nearly free)
  c[q] = lam * s1[q]/s2[q]  ->  transposed to a row (PE) and broadcast
    to all partitions (GPSIMD partition_broadcast)
  combine in SBUF: Ec = E1 - c*E2       (DVE, in-place, one pass)
  AV: O[q, d] = Ec.T @ V                (PE, single pass = half of v1)
  g[q] = rsqrt(mean_d(O^2) + eps*s1^2)  (ACT square-accum + ln/exp)
  osb = O * g  ->  transpose (PE)  ->  out = osb.T @ Wo'  -> DMA

Software pipeline per iteration t (one q tile):
  tailPE(t-2) | combines(t-1) on DVE | scores(t) with AV(t-1) interleaved
  on PE | c-chain(t) | tailpre(t-1).
"""

import numpy as np
import ml_dtypes
from collections import deque

import concourse.bass as bass
import concourse.mybir as mybir
import concourse.tile as tile
from concourse import bacc
from concourse.bass_utils import run_bass_kernel_spmd
from concourse.hw_specs import get_activation_tables
from concourse.masks import make_identity

B, S, E, H, D = 2, 2048, 512, 8, 512
HALF = D // 2
HLOC = 2            # heads per core
NCORES = 8
QT = 256            # q tile
NQT = S // QT       # 8
KC = 128            # k chunk
NKC = S // KC       # 16
NKP = NKC // 2      # k pairs
NQC = QT // 128     # 2
NDC = D // 128      # 4
NEC = E // 128      # 4
SCALE = 1.0 / float(np.sqrt(HALF))
EPS = float(np.finfo(np.float32).eps)
LAMBDA_INIT = 0.8

f32 = mybir.dt.float32
f32r = mybir.dt.float32r
bf16 = mybir.dt.float16  # fp16: same PE speed as bf16, 8x less rounding
AF = mybir.ActivationFunctionType
ALU = mybir.AluOpType

SKIP, FULL = -1, -2


def _analyze_mask(mask):
    """Per (q-tile, k-chunk) status: SKIP / FULL / keep-pattern index.

    A pattern is [128, 512]: the [128 k, 256 q] keep matrix duplicated
    across both halves.
    """
    status = [[SKIP] * NKC for _ in range(NQT)]
    pats = []
    pat_idx = {}
    for t in range(NQT):
        for kc in range(NKC):
            blk = mask[t * QT:(t + 1) * QT, kc * 128:(kc + 1) * 128]  # [256q, 128k]
            if blk.all():
                status[t][kc] = SKIP
            elif not blk.any():
                status[t][kc] = FULL
            else:
                keep = (~blk).T.astype(np.float32)  # [128, 256]
                pat = np.concatenate([keep, keep], axis=1)  # [128, 512]
                key = pat.tobytes()
                if key not in pat_idx:
                    pat_idx[key] = len(pats)
                    pats.append(pat)
                status[t][kc] = pat_idx[key]
    return status, pats


def _build(status, npat, repeat=1, unroll=1):  # noqa: C901
    nc = bacc.Bacc("TRN2", target_bir_lowering=False, debug=False)

    xt_d = nc.dram_tensor("xt", [E, S], bf16, kind="ExternalInput")
    wq_d = nc.dram_tensor("wq", [HLOC, E, D], bf16, kind="ExternalInput")
    wk_d = nc.dram_tensor("wk", [HLOC, E, D], bf16, kind="ExternalInput")
    wv_d = nc.dram_tensor("wv", [HLOC, E, D], bf16, kind="ExternalInput")
    wo_d = nc.dram_tensor("wo", [HLOC, D, E], bf16, kind="ExternalInput")
    lamneg_d = nc.dram_tensor("lamneg", [HLOC, 128, 1], f32, kind="ExternalInput")
    keeps_d = nc.dram_tensor("keeps", [max(npat, 1), 128, 512], bf16,
                             kind="ExternalInput")
    out_d = nc.dram_tensor("out", [HLOC, S, E], f32, kind="ExternalOutput")
    iters_d = nc.dram_tensor("iters", [1, 1], f32, kind="ExternalOutput") if repeat > 1 else None

    act_sets = list(get_activation_tables(nc.m.arch).keys())
    nle_set = act_sets.index("natural_log_exp_and_others")

    with tile.TileContext(nc) as tc:
        with tc.tile_pool(name="cst", bufs=1) as cst, \
             tc.tile_pool(name="big", bufs=1) as big, \
             tc.tile_pool(name="big2", bufs=2) as big2, \
             tc.tile_pool(name="epool", bufs=2) as epool, \
             tc.tile_pool(name="wts", bufs=2) as wts, \
             tc.tile_pool(name="scr", bufs=2) as scr, \
             tc.tile_pool(name="ps", bufs=3, space="PSUM") as ps, \
             tc.tile_pool(name="pst", bufs=2, space="PSUM") as pst, \
             tc.tile_pool(name="pso", bufs=1, space="PSUM") as pso, \
             tc.tile_pool(name="pss", bufs=1, space="PSUM") as pss:

            nc.scalar.add_instruction(mybir.InstLoadActFuncSet(
                name=nc.get_next_instruction_name(),
                ins=[], outs=[], act_func_set_id=nle_set))

            ident = cst.tile([128, 128], bf16, tag="ident")
            make_identity(nc, ident[:])
            ones_bf = cst.tile([128, 1], bf16, tag="ones")
            nc.gpsimd.memset(ones_bf[:], 1.0)
            keeps_t = cst.tile([128, max(npat, 1), 512], bf16, tag="keeps")
            for i in range(npat):
                nc.sync.dma_start(keeps_t[:, i, :], keeps_d.ap()[i])
            lam_t = cst.tile([128, HLOC], f32, tag="lam")
            for h in range(HLOC):
                nc.sync.dma_start(lam_t[:, h:h + 1], lamneg_d.ap()[h])

            # xT[e, s]: straight DMA (host pre-transposed), split per ec chunk
            # so the first projection matmuls can start before the full load
            xT = big.tile([128, NEC, S], bf16, tag="xT")
            for ec in range(NEC):
                nc.sync.dma_start(
                    xT[:, ec, :], xt_d.ap()[ec * 128:(ec + 1) * 128, :])

            if repeat > 1:
                ctr = cst.tile([1, 1], f32, tag="ctr")
                nc.gpsimd.memset(ctr[:], 0.0)
            rep_ctx = tc.For_i(0, repeat, 1) if repeat > 1 else None
            if rep_ctx is not None:
                rep_ctx.__enter__()
                nc.vector.tensor_scalar_add(ctr[:], ctr[:], 1.0)

            def emit_body():
                comb_q = deque()      # jobs awaiting c-chain-PE + combine + AV
                tailpe_q = deque()    # jobs awaiting transpose/outproj/DMA

                def emit_tailpe(j):
                    h = j["h"]
                    for qc in range(NQC):
                        tp_g = pst.tile([128, 512], bf16, tag="tail",
                                        name="tp_g")
                        for dc in range(NDC):
                            nc.tensor.transpose(
                                tp_g[:, dc * 128:(dc + 1) * 128],
                                j["osb"][qc][:, dc * 128:(dc + 1) * 128],
                                ident[:])
                        ot_t = scr.tile([128, NDC, 128], bf16, tag=f"ot{qc}")
                        nc.vector.tensor_copy(ot_t[:], tp_g[:])
                        out_ps = pst.tile([128, E], f32, tag="tail",
                                          name="out_ps")
                        for dc in range(NDC):
                            nc.tensor.matmul(
                                out_ps[:], ot_t[:, dc, :], j["wo"][:, dc, :],
                                start=(dc == 0), stop=(dc == NDC - 1))
                        out_sb = scr.tile([128, E], f32, tag="outsb")
                        nc.vector.tensor_copy(out_sb[:], out_ps[:])
                        q0 = j["t"] * QT + qc * 128
                        nc.sync.dma_start(out_d.ap()[h, q0:q0 + 128, :], out_sb[:])

                def emit_cchain_pe(j):
                    # ct transposes + partition broadcast for tile t-1;
                    # rec/cneg were computed at the end of the prior iteration
                    cs_t = scr.tile([1, QT], bf16, tag="cs")
                    for qc in range(NQC):
                        ct_ps = pst.tile([1, 128], bf16, tag="tail",
                                         name="ct_ps")
                        nc.tensor.transpose(
                            ct_ps[:], j["cneg"][:, qc:qc + 1], ident[:])
                        nc.vector.tensor_copy(
                            cs_t[:, qc * 128:(qc + 1) * 128], ct_ps[:])
                    cb = scr.tile([128, 1, QT], bf16, tag="cb")
                    nc.gpsimd.partition_broadcast(cb[:, 0, :], cs_t[:])
                    j["cb"] = cb

                def emit_combines(j):
                    Et, cb = j["E"], j["cb"]
                    nkc = len(j["kcs"])
                    for p in range(nkc // 2):
                        e2 = Et[:, 2 * p:2 * p + 2, 256:512]
                        nc.vector.tensor_tensor(
                            out=e2, in0=e2,
                            in1=cb[:].to_broadcast([128, 2, 256]), op=ALU.mult)
                        nc.vector.tensor_tensor(
                            out=Et[:, 2 * p:2 * p + 2, 0:256],
                            in0=Et[:, 2 * p:2 * p + 2, 0:256],
                            in1=e2, op=ALU.add)

                def emit_av_chunk(j, i):
                    Et, Vt = j["E"], j["V"]
                    kc = j["kcs"][i]
                    nkc = len(j["kcs"])
                    for qc in range(NQC):
                        nc.tensor.matmul(
                            j["o_ps"][:, qc * 512:(qc + 1) * 512],
                            Et[:, kc, qc * 128:(qc + 1) * 128],
                            Vt[:, kc, :],
                            start=(i == 0), stop=(i == nkc - 1))

                def emit_tailpre(j):
                    ms = scr.tile([128, NQC], f32, tag="ms")
                    lnm = scr.tile([128, NQC], f32, tag="lnm")
                    g = scr.tile([128, NQC], f32, tag="g")
                    osbs = []
                    for qc in range(NQC):
                        osq = scr.tile([128, D], bf16, tag="osq", bufs=1)
                        nc.scalar.activation(
                            osq[:], j["o_ps"][:, qc * 512:(qc + 1) * 512],
                            AF.Square, scale=float(1.0 / np.sqrt(D)),
                            accum_out=ms[:, qc:qc + 1])
                    # eps term dropped (validated: rel err 6.8e-4 -> 7.6e-4)
                    nc.scalar.activation(lnm[:], ms[:], AF.Ln)
                    nc.scalar.activation(g[:], lnm[:], AF.Exp, scale=-0.5)
                    for qc in range(NQC):
                        osb = scr.tile([128, D], bf16, tag=f"osb{qc}")
                        nc.vector.tensor_scalar_mul(
                            osb[:], j["o_ps"][:, qc * 512:(qc + 1) * 512],
                            g[:, qc:qc + 1])
                        osbs.append(osb)
                    j["osb"] = osbs

                def emit_iteration(hstate, t):
                    j = comb_q.popleft() if comb_q else None
                    if j is not None:
                        emit_combines(j)
                        j["o_ps"] = pso.tile([128, 2 * D], f32, tag="ops", name="o_ps")
                    nj = len(j["kcs"]) if j is not None else 0
                    popped_tail = False

                    if t is not None:
                        h = hstate["h"]
                        st = hstate["status"][t]
                        kcs = [kc for kc in range(NKC) if st[kc] != SKIP]
                        nkc = len(kcs)
                        q0 = t * QT
                        KTt, QTt = hstate["KT"], hstate["QT"]

                        Et = epool.tile([128, NKC, 512], bf16, tag="E")
                        sums_ps = pss.tile([128, 2 * NQC], f32, tag="sums")

                        def emit_sums(i):
                            # single PSUM accumulation group across all four
                            # (half, qc) columns: they share one zero region
                            kc = kcs[i]
                            for half in (0, 1):
                                for qc in range(NQC):
                                    nc.tensor.matmul(
                                        sums_ps[:, half * NQC + qc:
                                                half * NQC + qc + 1],
                                        Et[:, kc,
                                           half * 256 + qc * 128:
                                           half * 256 + qc * 128 + 128],
                                        ones_bf[:],
                                        start=(i == 0 and half == 0 and
                                               qc == 0),
                                        stop=(i == nkc - 1 and half == 1 and
                                              qc == NQC - 1))

                        for i, kc in enumerate(kcs):
                            sps = ps.tile([128, 512], f32, tag="mm",
                                          name="sps")
                            for half in (0, 1):
                                for jdc in (0, 1):
                                    dc = half * 2 + jdc
                                    nc.tensor.matmul(
                                        sps[:, half * 256:half * 256 + 256],
                                        KTt[:, dc, kc * 128:(kc + 1) * 128],
                                        QTt[:, dc, q0:q0 + QT],
                                        start=(jdc == 0), stop=(jdc == 1))
                            nc.scalar.activation(
                                Et[:, kc, :], sps[:], AF.Exp, scale=SCALE)
                            pat = st[kc]
                            if pat >= 0:
                                nc.vector.tensor_tensor(
                                    out=Et[:, kc, :], in0=Et[:, kc, :],
                                    in1=keeps_t[:, pat, :], op=ALU.mult)
                            if i > 0:
                                emit_sums(i - 1)
                            if j is not None and 1 <= i <= nj:
                                emit_av_chunk(j, i - 1)
                            if tailpe_q and i == min(3, nkc - 1):
                                emit_tailpe(tailpe_q.popleft())
                                popped_tail = True
                        emit_sums(nkc - 1)
                        # rec/cneg right away (DVE) while PE drains AVs
                        rec = scr.tile([128, NQC], f32, tag="rec")
                        nc.vector.reciprocal(rec[:], sums_ps[:, NQC:2 * NQC])
                        cneg = scr.tile([128, NQC], bf16, tag="cneg")
                        nc.vector.scalar_tensor_tensor(
                            out=cneg[:], in0=sums_ps[:, 0:NQC],
                            scalar=lam_t[:, h:h + 1],
                            in1=rec[:], op0=ALU.mult, op1=ALU.mult)
                        if j is not None:
                            for i2 in range(max(0, min(nkc - 1, nj)), nj):
                                emit_av_chunk(j, i2)

                        job = dict(
                            h=h, t=t, E=Et, V=hstate["V"], wo=hstate["wo"],
                            cneg=cneg, kcs=kcs)
                        emit_cchain_pe(job)
                        comb_q.append(job)
                    else:
                        if tailpe_q:
                            emit_tailpe(tailpe_q.popleft())
                            popped_tail = True
                        if j is not None:
                            for i2 in range(nj):
                                emit_av_chunk(j, i2)

                    if tailpe_q and not popped_tail:
                        emit_tailpe(tailpe_q.popleft())
                    if j is not None:
                        emit_tailpre(j)
                        tailpe_q.append(j)

                for h in range(HLOC):
                    wq_t = wts.tile([128, NEC, D], bf16, tag="wq")
                    wk_t = wts.tile([128, NEC, D], bf16, tag="wk")
                    wv_t = wts.tile([128, NEC, D], bf16, tag="wv")
                    wo_t = wts.tile([128, NDC, E], bf16, tag="wo")
                    for ec in range(NEC):
                        nc.sync.dma_start(wq_t[:, ec, :], wq_d.ap()[h, ec * 128:(ec + 1) * 128, :])
                        nc.sync.dma_start(wk_t[:, ec, :], wk_d.ap()[h, ec * 128:(ec + 1) * 128, :])
                        nc.sync.dma_start(wv_t[:, ec, :], wv_d.ap()[h, ec * 128:(ec + 1) * 128, :])
                    for dc in range(NDC):
                        nc.sync.dma_start(wo_t[:, dc, :], wo_d.ap()[h, dc * 128:(dc + 1) * 128, :])

                    # KT[d, k] = Wk.T @ xT ; QT[d, q] = Wq.T @ xT
                    KTt = big.tile([128, NDC, S], bf16, tag="KT")
                    QTt = big.tile([128, NDC, S], bf16, tag="QT")
                    for (w_t, dst, cp) in ((wk_t, KTt, "k"), (wq_t, QTt, "q")):
                        for dc in range(NDC):
                            for kt in range(S // 512):
                                kps = ps.tile([128, 512], f32, tag="mm",
                                              name="kps")
                                for ec in range(NEC):
                                    nc.tensor.matmul(
                                        kps[:],
                                        w_t[:, ec, dc * 128:(dc + 1) * 128],
                                        xT[:, ec, kt * 512:(kt + 1) * 512],
                                        start=(ec == 0), stop=(ec == NEC - 1))
                                if (dc + kt) % 2 == 0:
                                    nc.vector.tensor_copy(
                                        dst[:, dc, kt * 512:(kt + 1) * 512],
                                        kps[:])
                                else:
                                    nc.scalar.activation(
                                        dst[:, dc, kt * 512:(kt + 1) * 512],
                                        kps[:], AF.Copy)

                    # V[s, d] = x @ Wv
                    Vt = big2.tile([128, NKC, D], bf16, tag="V")
                    for sc in range(NKC):
                        vps = ps.tile([128, 512], f32, tag="mm", name="vps")
                        for ec in range(NEC):
                            nc.tensor.matmul(
                                vps[:],
                                xT[:, ec, sc * 128:(sc + 1) * 128],
                                wv_t[:, ec, :],
                                start=(ec == 0), stop=(ec == NEC - 1))
                        if sc % 2 == 0:
                            nc.vector.tensor_copy(Vt[:, sc, :], vps[:])
                        else:
                            nc.scalar.activation(Vt[:, sc, :], vps[:], AF.Copy)

                    hstate = dict(h=h, status=status, KT=KTt, QT=QTt, V=Vt,
                                  wo=wo_t)
                    for t in range(NQT):
                        emit_iteration(hstate, t)
                emit_iteration(None, None)
                emit_iteration(None, None)

            for _u in range(unroll):
                emit_body()

            if rep_ctx is not None:
                rep_ctx.__exit__(None, None, None)
                nc.sync.dma_start(iters_d.ap()[:], ctr[:])

    nc.compile()
    return nc


_CACHE = {}


def _get_program(mask, repeat=1, unroll=1):
    key = (mask.tobytes(), repeat, unroll)
    if key not in _CACHE:
        status, pats = _analyze_mask(mask)
        nc = _build(status, len(pats), repeat=repeat, unroll=unroll)
        _CACHE[key] = (nc, pats)
    return _CACHE[key]


def make_in_maps(x, mask, Wq, bq, Wk, bk, Wv, bv, lq1, lk1, lq2, lk2,
                 lam_init_p, rms_w, Wo, bo, repeat=1, unroll=1):
    x = np.asarray(x, np.float32)
    mask = np.asarray(mask, bool)
    Wq = np.asarray(Wq, np.float32)
    Wk = np.asarray(Wk, np.float32)
    Wv = np.asarray(Wv, np.float32)
    Wo = np.asarray(Wo, np.float32)
    for b_ in (bq, bk, bv):
        assert np.abs(np.asarray(b_)).max() == 0.0, "nonzero qkv bias unsupported"
    lam = (np.exp((np.asarray(lq1, np.float32) * np.asarray(lk1, np.float32)).sum(-1))
           - np.exp((np.asarray(lq2, np.float32) * np.asarray(lk2, np.float32)).sum(-1))
           + np.asarray(lam_init_p, np.float32))  # [H]
    woF = Wo.reshape(H, D, E) * ((1.0 - LAMBDA_INIT) * np.asarray(rms_w, np.float32))[:, :, None]

    nc, pats = _get_program(mask, repeat=repeat, unroll=unroll)
    if pats:
        keeps = np.stack(pats).astype(np.float16)
    else:
        keeps = np.zeros((1, 128, 512), np.float16)

    in_maps = []
    for c in range(NCORES):
        b = c // 4
        h0 = HLOC * (c % 4)
        lamneg = np.repeat((-lam[h0:h0 + HLOC]).astype(np.float32)[:, None, None], 128, axis=1)
        in_maps.append({
            "xt": np.ascontiguousarray(x[b].T).astype(np.float16),
            "wq": np.ascontiguousarray(Wq[h0:h0 + HLOC]).astype(np.float16),
            "wk": np.ascontiguousarray(Wk[h0:h0 + HLOC]).astype(np.float16),
            "wv": np.ascontiguousarray(Wv[h0:h0 + HLOC]).astype(np.float16),
            "wo": np.ascontiguousarray(woF[h0:h0 + HLOC]).astype(np.float16),
            "lamneg": np.ascontiguousarray(lamneg),
            "keeps": keeps,
        })
    return nc, in_maps


def gather(results, bo):
    out = np.zeros((B, S, E), np.float32)
    for c in range(NCORES):
        out[c // 4] += results[c]["out"].sum(axis=0)
    out += np.asarray(bo, np.float32)[None, None, :]
    return out


def kernel(**inputs):
    nc, in_maps = make_in_maps(**inputs)
    res = run_bass_kernel_spmd(nc, in_maps, core_ids=list(range(NCORES)))
    return gather(res.results, inputs["bo"])

